# revision 4
# baseline (speedup 1.0000x reference)
"""LoRA grouped-experts MoE MLP on 8 NeuronCores (expert-parallel).

Each core computes one expert's full MLP:
    g = silu(x @ Wg + (x @ Ag) @ (s*Bg))
    u =       x @ Wu + (x @ Au) @ (s*Bu)
    h = g * u
    o =       h @ Wd + (h @ Ad) @ (s*Bd)

Device layout (per core):
  - x is pre-transposed on host to xT [D, T] so the contraction dim D lands
    on SBUF partitions for both matmul operands (fp32 has no DMA transpose).
  - Layer 1 computes hT [H, T] (H on partitions); layer 2 then uses hT
    slices as the stationary operand, producing out [T, D] directly.
  - All matmul inputs are bf16 (cast on host); PSUM accumulates fp32.
  - LoRA rank padded 16->32; lora B pre-scaled by alpha/rank. The LoRA
    contribution is accumulated into the same PSUM group as the base matmul.
"""

import os

import numpy as np
import ml_dtypes

import concourse.bacc as bacc
import concourse.mybir as mybir
import concourse.tile as tile
from concourse.bass import ts
from concourse.bass_utils import run_bass_kernel_spmd

P = 128
E, D, H, R, T = 8, 2048, 4096, 16, 1024
RP = 32  # padded lora rank (K>=32 for PE matmuls)
DO = D // P   # 16
HO = H // P   # 32
ALPHA = 32.0
BF16 = mybir.dt.bfloat16
F32 = mybir.dt.float32

_NC_CACHE = []
LAST_RESULT = None


def _build_nc():
    nc = bacc.Bacc("TRN2", target_bir_lowering=False, debug=False, num_devices=E)

    xT = nc.dram_tensor("xT", (D, T), BF16, kind="ExternalInput").ap()
    wg = nc.dram_tensor("wg", (D, H), BF16, kind="ExternalInput").ap()
    wu = nc.dram_tensor("wu", (D, H), BF16, kind="ExternalInput").ap()
    wd = nc.dram_tensor("wd", (H, D), BF16, kind="ExternalInput").ap()
    ag = nc.dram_tensor("ag", (D, RP), BF16, kind="ExternalInput").ap()
    bg = nc.dram_tensor("bg", (RP, H), BF16, kind="ExternalInput").ap()
    au = nc.dram_tensor("au", (D, RP), BF16, kind="ExternalInput").ap()
    bu = nc.dram_tensor("bu", (RP, H), BF16, kind="ExternalInput").ap()
    ad = nc.dram_tensor("ad", (H, RP), BF16, kind="ExternalInput").ap()
    bd = nc.dram_tensor("bd", (RP, D), BF16, kind="ExternalInput").ap()
    out = nc.dram_tensor("out", (T, D), F32, kind="ExternalOutput").ap()

    xT_r = xT.rearrange("(o p) t -> p o t", p=P)   # [128, 16, 1024]
    wg_r = wg.rearrange("(o p) h -> p o h", p=P)   # [128, 16, 4096]
    wu_r = wu.rearrange("(o p) h -> p o h", p=P)
    wd_r = wd.rearrange("(o p) d -> p o d", p=P)   # [128, 32, 2048]
    ag_r = ag.rearrange("(o p) r -> p o r", p=P)   # [128, 16, 32]
    au_r = au.rearrange("(o p) r -> p o r", p=P)
    ad_r = ad.rearrange("(o p) r -> p o r", p=P)   # [128, 32, 32]

    TQ = T // 512  # 2

    with tile.TileContext(nc) as tc:
        with (
            tc.tile_pool(name="persist", bufs=1) as pp,
            tc.tile_pool(name="wpool", bufs=3) as wp,
            tc.tile_pool(name="stage", bufs=2) as sp,
            tc.tile_pool(name="psum", bufs=2, space="PSUM") as psp,
        ):
            xT_sb = pp.tile([P, DO, T], BF16)
            hT_sb = pp.tile([P, HO, T], BF16)
            ag_sb = pp.tile([P, DO, RP], BF16)
            au_sb = pp.tile([P, DO, RP], BF16)
            ad_sb = pp.tile([P, HO, RP], BF16)
            bg_sb = pp.tile([RP, H], BF16)
            bu_sb = pp.tile([RP, H], BF16)
            bd_sb = pp.tile([RP, D], BF16)
            aTg_sb = pp.tile([RP, T], BF16)
            aTu_sb = pp.tile([RP, T], BF16)
            aTd_sb = pp.tile([RP, T], BF16)

            nc.sync.dma_start(xT_sb[:], xT_r[:])
            nc.sync.dma_start(ag_sb[:], ag_r[:])
            nc.sync.dma_start(au_sb[:], au_r[:])
            nc.sync.dma_start(ad_sb[:], ad_r[:])
            nc.sync.dma_start(bg_sb[:], bg[:])
            nc.sync.dma_start(bu_sb[:], bu[:])
            nc.sync.dma_start(bd_sb[:], bd[:])

            # aT = (x @ A)^T for gate/up (scale folded into B on host)
            for a_sb, aT_sb, nm in ((ag_sb, aTg_sb, "g"), (au_sb, aTu_sb, "u")):
                for t in range(TQ):
                    pa = psp.tile([RP, 512], F32, tag="pa")
                    for o in range(DO):
                        nc.tensor.matmul(
                            pa[:], a_sb[:, o, :], xT_sb[:, o, ts(t, 512)],
                            start=(o == 0), stop=(o == DO - 1),
                        )
                    nc.vector.tensor_copy(aT_sb[:, ts(t, 512)], pa[:])

            # layer 1: hT[h, t] = silu(gate) * up
            for j in range(H // 512):
                wg_t = wp.tile([P, DO, 512], BF16, tag="w")
                nc.sync.dma_start(wg_t[:], wg_r[:, :, ts(j, 512)])
                wu_t = wp.tile([P, DO, 512], BF16, tag="w")
                nc.sync.dma_start(wu_t[:], wu_r[:, :, ts(j, 512)])
                for hsub in range(4):
                    hc = j * 4 + hsub
                    for t in range(TQ):
                        pg = psp.tile([P, 512], F32, tag="pg")
                        for o in range(DO):
                            nc.tensor.matmul(
                                pg[:], wg_t[:, o, ts(hsub, P)],
                                xT_sb[:, o, ts(t, 512)],
                                start=(o == 0), stop=False,
                            )
                        nc.tensor.matmul(
                            pg[:], bg_sb[:, ts(hc, P)], aTg_sb[:, ts(t, 512)],
                            start=False, stop=True,
                        )
                        pu = psp.tile([P, 512], F32, tag="pu")
                        for o in range(DO):
                            nc.tensor.matmul(
                                pu[:], wu_t[:, o, ts(hsub, P)],
                                xT_sb[:, o, ts(t, 512)],
                                start=(o == 0), stop=False,
                            )
                        nc.tensor.matmul(
                            pu[:], bu_sb[:, ts(hc, P)], aTu_sb[:, ts(t, 512)],
                            start=False, stop=True,
                        )
                        g_act = sp.tile([P, 512], F32, tag="gact")
                        nc.scalar.activation(
                            g_act[:], pg[:], mybir.ActivationFunctionType.Silu
                        )
                        nc.vector.tensor_mul(
                            hT_sb[:, hc, ts(t, 512)], g_act[:], pu[:]
                        )

            # aTd = (h @ Ad)^T
            for t in range(TQ):
                pa = psp.tile([RP, 512], F32, tag="pa")
                for hc in range(HO):
                    nc.tensor.matmul(
                        pa[:], ad_sb[:, hc, :], hT_sb[:, hc, ts(t, 512)],
                        start=(hc == 0), stop=(hc == HO - 1),
                    )
                nc.vector.tensor_copy(aTd_sb[:, ts(t, 512)], pa[:])

            # layer 2: out[t, d] = h @ Wd + lora
            for k in range(D // 512):
                wd_t0 = wp.tile([P, DO, 512], BF16, tag="w")
                nc.sync.dma_start(wd_t0[:], wd_r[:, 0:16, ts(k, 512)])
                wd_t1 = wp.tile([P, DO, 512], BF16, tag="w")
                nc.sync.dma_start(wd_t1[:], wd_r[:, 16:32, ts(k, 512)])
                for tt in range(T // P):
                    po = psp.tile([P, 512], F32, tag="po")
                    for hc in range(HO):
                        w_t = wd_t0 if hc < 16 else wd_t1
                        nc.tensor.matmul(
                            po[:], hT_sb[:, hc, ts(tt, P)], w_t[:, hc % 16, :],
                            start=(hc == 0), stop=False,
                        )
                    nc.tensor.matmul(
                        po[:], aTd_sb[:, ts(tt, P)], bd_sb[:, ts(k, 512)],
                        start=False, stop=True,
                    )
                    o_t = sp.tile([P, 512], F32, tag="ostage")
                    nc.scalar.copy(o_t[:], po[:])
                    nc.sync.dma_start(out[ts(tt, P), ts(k, 512)], o_t[:])

    nc.compile()
    return nc


def _get_nc():
    if not _NC_CACHE:
        _NC_CACHE.append(_build_nc())
    return _NC_CACHE[0]


def kernel(x, num_tokens_per_expert, gate_proj, up_proj, down_proj,
           lora_gate_a, lora_gate_b, lora_up_a, lora_up_b,
           lora_down_a, lora_down_b):
    global LAST_RESULT
    in_maps = make_in_maps(x, gate_proj, up_proj, down_proj,
                           lora_gate_a, lora_gate_b, lora_up_a, lora_up_b,
                           lora_down_a, lora_down_b)
    # The axon NTFF profile hook is unavailable in this container; force the
    # no-trace PJRT path regardless of ambient BASS_TRACE.
    os.environ["BASS_NEVER_TRACE"] = "1"
    nc = _get_nc()
    res = run_bass_kernel_spmd(nc, in_maps, core_ids=list(range(E)))
    LAST_RESULT = res
    return np.concatenate([r["out"] for r in res.results], axis=0)


def make_in_maps(x, gate_proj, up_proj, down_proj, lga, lgb, lua, lub, lda, ldb):
    """Host-side shard/cast prep, shared by kernel() and the bench harness."""
    bf = ml_dtypes.bfloat16
    scale = ALPHA / R
    x = np.asarray(x, np.float32).reshape(E, T, D)

    def pad_a(a):
        o = np.zeros((a.shape[0], RP), np.float32)
        o[:, :R] = a
        return o.astype(bf)

    def pad_b(b):
        o = np.zeros((RP, b.shape[1]), np.float32)
        o[:R] = scale * b
        return o.astype(bf)

    in_maps = []
    for e in range(E):
        in_maps.append({
            "xT": np.ascontiguousarray(x[e].T).astype(bf),
            "wg": np.asarray(gate_proj[e], np.float32).astype(bf),
            "wu": np.asarray(up_proj[e], np.float32).astype(bf),
            "wd": np.asarray(down_proj[e], np.float32).astype(bf),
            "ag": pad_a(np.asarray(lga[e], np.float32)),
            "bg": pad_b(np.asarray(lgb[e], np.float32)),
            "au": pad_a(np.asarray(lua[e], np.float32)),
            "bu": pad_b(np.asarray(lub[e], np.float32)),
            "ad": pad_a(np.asarray(lda[e], np.float32)),
            "bd": pad_b(np.asarray(ldb[e], np.float32)),
        })
    return in_maps
